# revision 12
# baseline (speedup 1.0000x reference)
"""Causal BoW (running mean over T) Trainium2 kernel.

out[b, t, c] = sum_{s<=t} x[b, s, c] / (t+1)   for x of shape [32, 2048, 512] f32.

Sharding: batch B=32 across 8 NeuronCores (4 samples each), no cross-core comms.

Per-core algorithm (per sample [T=2048, C=512], 16 T-blocks of 128 rows):
  - Single f32r matmul per block (f32r streams 1 cycle/row vs 4 for f32;
    11-bit mantissa gives ~1e-4 rel err, far inside the 2e-2 gate, so no
    hi/lo split): psum_j = U128^T.T @ x_j with U128 = upper-triangular ones.
  - Block offsets off[m, c] = sum_{k<m} colsum(x_k)[c] via 15 accumulating
    "step" matmuls (step_k[p, m] = 1 if m > k) into one [16, 512] PSUM bank.
  - Offset injection: off[j] is ADDED to row 0 of block j by one SWDGE
    SBUF->SBUF DMA with accum_op=add (CCE inline adder). The block scan
    then propagates it to every row: L @ (x_j + e0*off_j) = cumsum + off_j.
    This removes the per-block broadcast matmuls entirely. Block 0 needs no
    offset and is excluded from the scatter so its scan never waits on it.
  - Eviction: per-partition scale recip[p, j] = 1/(j*128+p+1) applied while
    moving PSUM -> SBUF (f16 out), alternating DVE tensor_scalar_mul
    ((120+512)/0.96 ~ 658 ns) and ACT activation-mul ((172+512)/1.2 ~ 570
    ns): fp32 PSUM reads are 1x on both engines, and one engine alone
    cannot feed the store stream at line rate.
  - y is stored as f16 (halves store traffic; f16 round-off ~5e-4 rel vs
    the 2e-2 gate) and upcast to f32 on the host after the gather.
  - DMA ring split: x loads issue on nc.sync (qSPDynamicHW), y stores on
    nc.scalar (qActDynamicHW). HWDGE rings are FIFO per issuing engine, so
    a store blocked on eviction can never head-of-line-block a load; the 16
    SDMA engines round-robin the two rings at packet granularity.
  - xpool bufs=4 so all four samples' loads queue immediately: the load
    stream saturates HBM from the start instead of stalling on xt reuse.
  - All DMAs keep full 128-partition access patterns with >=1 KB descriptors.
"""

import numpy as np

import concourse.bass as bass
import concourse.bacc as bacc
import concourse.mybir as mybir
from concourse import tile
from concourse.bass_utils import run_bass_kernel_spmd

B, T, C = 32, 2048, 512
N_CORES = 8
BS = B // N_CORES          # samples per core
P = 128                    # partitions / T-block size
NBLK = T // P              # 16 blocks per sample
NQ = 4                     # DMA chunks per sample (1 MB each)
NH = NBLK // NQ            # blocks per chunk (4)
F32 = mybir.dt.float32
F32R = mybir.dt.float32r
F16 = mybir.dt.float16

_cache = {}


def _build():
    nc = bacc.Bacc()
    x = nc.dram_tensor("x", [BS, T, C], F32R, kind="ExternalInput")
    u128 = nc.dram_tensor("u128", [P, P], F32R, kind="ExternalInput")
    stepm = nc.dram_tensor("stepm", [P, NBLK * NBLK], F32R, kind="ExternalInput")
    recip = nc.dram_tensor("recip", [P, NBLK], F32, kind="ExternalInput")
    y = nc.dram_tensor("y", [BS, T, C], F16, kind="ExternalOutput")

    with tile.TileContext(nc) as tc:
        with (
            tc.tile_pool(name="singles", bufs=1) as singles,
            tc.tile_pool(name="xp", bufs=4) as xpool,
            tc.tile_pool(name="yp", bufs=3) as ypool,
            tc.tile_pool(name="offp", bufs=2) as offpool,
            tc.tile_pool(name="pblk", bufs=6, space="PSUM") as pblk,
            tc.tile_pool(name="poff", bufs=2, space="PSUM") as poff,
        ):
            u_t = singles.tile([P, P], F32R)
            nc.scalar.dma_start(out=u_t[:], in_=u128[:])
            step_t = singles.tile([P, NBLK * NBLK], F32R)
            nc.scalar.dma_start(out=step_t[:], in_=stepm[:])
            recip_t = singles.tile([P, NBLK], F32)
            nc.scalar.dma_start(out=recip_t[:], in_=recip[:])

            for b in range(BS):
                xs = x[b].rearrange("(j p) c -> p j c", p=P)   # [128, 16, 512]
                ys = y[b].rearrange("(j p) c -> p j c", p=P)

                xt = xpool.tile([P, NBLK * C], F32R, tag="xt", name="xt")
                xt3 = xt.rearrange("p (j c) -> p j c", c=C)
                for h in range(NQ):
                    nc.sync.dma_start(
                        out=xt3[:, h * NH:(h + 1) * NH, :],
                        in_=xs[:, h * NH:(h + 1) * NH, :],
                    )

                # off[m, c] = sum_{k<m} (block-k column sum); k=15 feeds no m
                offp_t = poff.tile([NBLK, C], F32)
                for k in range(NBLK - 1):
                    sel = step_t[:, k * NBLK:(k + 1) * NBLK]
                    nc.tensor.matmul(
                        offp_t[:],
                        sel,
                        xt[:, k * C:(k + 1) * C],
                        start=(k == 0),
                        stop=(k == NBLK - 2),
                    )
                off_sb = offpool.tile([NBLK, C], F32R, tag="off")
                nc.scalar.copy(out=off_sb[:], in_=offp_t[:])

                # scatter-accumulate off[j] into row 0 of block j (j >= 1);
                # the scan matmul then carries it into every row of the block
                nc.gpsimd.dma_start(
                    out=xt3[0:1, 1:NBLK, :],
                    in_=off_sb[1:NBLK, :],
                    accum_op=mybir.AluOpType.add,
                )

                yt = ypool.tile([P, NBLK * C], F16, tag="yt", name="yt")
                yt3 = yt.rearrange("p (j c) -> p j c", c=C)
                for h in range(NQ):
                    for jj in range(NH):
                        j = h * NH + jj
                        cs = slice(j * C, (j + 1) * C)
                        pb = pblk.tile([P, C], F32)
                        nc.tensor.matmul(pb[:], u_t[:], xt[:, cs],
                                         start=True, stop=True)
                        if jj % 2 == 0:
                            nc.vector.tensor_scalar_mul(
                                yt[:, cs], pb[:], recip_t[:, j:j + 1]
                            )
                        else:
                            nc.scalar.mul(
                                yt[:, cs], pb[:], recip_t[:, j:j + 1]
                            )
                    nc.scalar.dma_start(
                        out=ys[:, h * NH:(h + 1) * NH, :],
                        in_=yt3[:, h * NH:(h + 1) * NH, :],
                    )
    nc.finalize()
    return nc


def _consts():
    u = np.triu(np.ones((P, P), dtype=np.float32))
    step = np.zeros((P, NBLK * NBLK), dtype=np.float32)
    for k in range(NBLK):
        for m in range(NBLK):
            if m > k:
                step[:, k * NBLK + m] = 1.0
    recip = (1.0 / np.arange(1, T + 1, dtype=np.float32)).reshape(NBLK, P).T.copy()
    return u, step, recip


def run(x, trace=False):
    x = np.ascontiguousarray(np.asarray(x, dtype=np.float32))
    assert x.shape == (B, T, C), x.shape
    if "nc" not in _cache:
        _cache["nc"] = _build()
    nc = _cache["nc"]
    u, step, recip = _consts()
    in_maps = [
        {
            "x": np.ascontiguousarray(x[i * BS:(i + 1) * BS]),
            "u128": u,
            "stepm": step,
            "recip": recip,
        }
        for i in range(N_CORES)
    ]
    res = run_bass_kernel_spmd(nc, in_maps, list(range(N_CORES)), trace=trace)
    y = np.concatenate(
        [res.results[i]["y"].astype(np.float32) for i in range(N_CORES)], axis=0
    )
    return y, res.exec_time_ns


def kernel(x):
    y, _ = run(x, trace=False)
    return y
